# revision 1
# baseline (speedup 1.0000x reference)
"""Trainium2 Bass kernel for nn_CustomMultiresLayer (B=2, D=1024, L=4096, FS=4).

Sharding (8 cores): core c -> batch beta=c//4, channel shard gamma=c%4
(256 channels). Phase A computes the depthwise multires tree + gated
combination for the core's 256 channels. An AllGather within each 4-core
batch group assembles the full gated tensor y; each core then computes
its 256 OUTPUT channels of the 1x1 channel mix (w_mix @ y) over all 4096
positions, adds the residual, computes partial LayerNorm channel sums,
AllReduces the tiny [2,4096] stats, and normalizes its channel slab.

Engine plan per tree level: a-chain convs as diagonal-weight matmuls on
the tensor engine (bf16 in, fp32 PSUM accumulate), b convs split between
PE and DVE, sigmoid on ACT, gating mul/add on DVE. Channel mix: bf16
matmul, fp32 PSUM. LN stats via fp32r ones-matmuls over partitions,
normalization via fp32r outer-product scale/shift matrices.
"""

import numpy as np
import ml_dtypes

import concourse.bacc as bacc
import concourse.mybir as mybir
import concourse.tile as tile
from concourse.bass_utils import run_bass_kernel_spmd

F32 = mybir.dt.float32
F32R = mybir.dt.float32r
BF16 = mybir.dt.bfloat16
AF = mybir.ActivationFunctionType
ALU = mybir.AluOpType

B, D, L = 2, 1024, 4096
FS, DEPTH = 4, 11
LN_EPS = 1e-5
NC = 8
CH = 256          # channels per core (2 half-tiles of 128)
NMM = 512         # matmul moving-dim tile
GROUPS = [[0, 1, 2, 3], [4, 5, 6, 7]]

# which levels run each conv family on PE (rest on DVE)
PE_A_LEVELS = frozenset()
PE_B_LEVELS = frozenset()

_CACHE = {}


def _emit_conv_pe(nc, ps_pool, dst, src, diag, dil, engine):
    """4-tap dilated causal depthwise conv via diagonal-weight matmuls.
    dst, src: SBUF bf16 [128, L]; diag: SBUF bf16 [128, 4*128]."""
    for c0 in range(0, L, NMM):
        taps = []
        for k in (3, 2, 1, 0):
            s = (3 - k) * dil
            lo = max(0, s - c0)
            if lo < NMM:
                taps.append((k, s, lo))
        pp = ps_pool.tile([128, NMM], F32, tag="cps")
        for i, (k, s, lo) in enumerate(taps):
            nc.tensor.matmul(
                pp[:, lo:NMM],
                diag[:, 128 * k : 128 * (k + 1)],
                src[:, c0 + lo - s : c0 + NMM - s],
                start=(i == 0),
                stop=(i == len(taps) - 1),
            )
        if engine == "act":
            nc.scalar.copy(dst[:, c0 : c0 + NMM], pp[:])
        else:
            nc.vector.tensor_copy(dst[:, c0 : c0 + NMM], pp[:])


def _emit_conv_dve(nc, dst, src, h, dil, zb=None):
    """4-tap conv: tap-3 scaled copy on ACT (frees DVE), 3 MACs on DVE."""
    if zb is None:
        nc.vector.tensor_scalar_mul(dst[:], src[:], h[:, 3:4])
    else:
        nc.scalar.activation(
            dst[:], src[:], AF.Identity, bias=zb[:], scale=h[:, 3:4]
        )
    for k in (2, 1, 0):
        s = (3 - k) * dil
        if s < L:
            nc.vector.scalar_tensor_tensor(
                dst[:, s:L], src[:, 0 : L - s], h[:, k : k + 1], dst[:, s:L],
                ALU.mult, ALU.add,
            )


def _build_program(repeats: int = 1, comm: bool = True, do_tree: bool = True, do_pb: bool = True):
    nc = bacc.Bacc("TRN2", target_bir_lowering=False, debug=False, num_devices=NC)

    xs = nc.dram_tensor("xs", [CH, L], F32, kind="ExternalInput").ap()
    h0s = nc.dram_tensor("h0s", [CH, FS], F32, kind="ExternalInput").ap()
    h1s = nc.dram_tensor("h1s", [CH, FS], F32, kind="ExternalInput").ap()
    d0 = nc.dram_tensor("d0", [2, FS, 128, 128], BF16, kind="ExternalInput").ap()
    d1 = nc.dram_tensor("d1", [2, FS, 128, 128], BF16, kind="ExternalInput").ap()
    wTs = nc.dram_tensor("wTs", [D, CH], BF16, kind="ExternalInput").ap()
    bmixs = nc.dram_tensor("bmixs", [CH, 1], F32, kind="ExternalInput").ap()
    gams = nc.dram_tensor("gams", [1, CH], F32, kind="ExternalInput").ap()
    bets = nc.dram_tensor("bets", [1, CH], F32, kind="ExternalInput").ap()
    og = nc.dram_tensor("og", [CH, L], F32, kind="ExternalOutput").ap()

    with tile.TileContext(nc) as tc:
        for _rep in range(repeats):
            with (
                tc.tile_pool(name="dram", bufs=1, space="DRAM") as dram,
                tc.tile_pool(name="smalls", bufs=1) as smalls,
            ):
                y_loc = dram.tile([CH, L], BF16)
                y_gat = dram.tile([D, L], BF16)
                st_loc = dram.tile([2, L], F32)
                st_glb = dram.tile([2, L], F32)

                # ---------------- Phase A: multires tree ----------------
                with (
                    tc.tile_pool(name="tree", bufs=1) as tp,
                    tc.tile_pool(name="cpsum", bufs=6, space="PSUM") as cps,
                ):
                    a_t = [[tp.tile([128, L], F32, tag=f"a{h}{i}", name=f"a{h}{i}") for i in range(2)]
                           for h in range(2)]
                    b_t = [[tp.tile([128, L], F32, tag=f"b{h}{i}", name=f"b{h}{i}") for i in range(2)]
                           for h in range(2)]
                    sig_sh = tp.tile([128, L], F32, tag="sigsh", name="sigsh")
                    sig = [sig_sh, sig_sh]
                    y_t = [tp.tile([128, L], F32, tag=f"y{h}", name=f"y{h}") for h in range(2)]
                    y16 = [tp.tile([128, L], BF16, tag=f"y16{h}", name=f"y16{h}") for h in range(2)]
                    zb = smalls.tile([128, 1], F32, tag="zb", name="zb")
                    nc.vector.memset(zb[:], 0.0)
                    h0c = [smalls.tile([128, FS], F32, tag=f"h0c{h}", name=f"h0c{h}") for h in range(2)]
                    h1c = [smalls.tile([128, FS], F32, tag=f"h1c{h}", name=f"h1c{h}") for h in range(2)]
                    d0c = [smalls.tile([128, FS * 128], BF16, tag=f"d0c{h}", name=f"d0c{h}") for h in range(2)]
                    d1c = [smalls.tile([128, FS * 128], BF16, tag=f"d1c{h}", name=f"d1c{h}") for h in range(2)]

                    with tc.tile_pool(name="stage", bufs=2) as stage:
                        for h in range(2):
                            rs = slice(128 * h, 128 * (h + 1))
                            nc.sync.dma_start(a_t[h][0][:], xs[rs, :])
                            nc.sync.dma_start(h0c[h][:], h0s[rs, :])
                            nc.sync.dma_start(h1c[h][:], h1s[rs, :])
                            for k in range(FS):
                                ks = slice(128 * k, 128 * (k + 1))
                                nc.sync.dma_start(d0c[h][:, ks], d0[h, k])
                                nc.sync.dma_start(d1c[h][:, ks], d1[h, k])

                    for lvl in range(DEPTH if do_tree else 2):
                        dil = 1 << lvl
                        for h in range(2):
                            a_cur = a_t[h][lvl % 2]
                            a_nxt = a_t[h][(lvl + 1) % 2]
                            b_cur = b_t[h][lvl % 2]
                            b_prv = b_t[h][(lvl + 1) % 2]
                            if lvl in PE_A_LEVELS:
                                _emit_conv_pe(nc, cps, a_nxt, a_cur, d0c[h], dil, "act")
                            else:
                                _emit_conv_dve(nc, a_nxt, a_cur, h0c[h], dil, zb)
                            if lvl < DEPTH - 1:
                                if lvl in PE_B_LEVELS:
                                    _emit_conv_pe(nc, cps, b_cur, a_cur, d1c[h], dil, "dve")
                                else:
                                    _emit_conv_dve(nc, b_cur, a_cur, h1c[h], dil, zb)
                            if lvl >= 1:
                                nc.scalar.activation(sig[h][:], a_nxt[:], AF.Sigmoid)
                                nc.gpsimd.tensor_mul(sig[h][:], sig[h][:], b_prv[:])
                                if lvl == 1:
                                    nc.vector.tensor_scalar_mul(y_t[h][:], sig[h][:], 2.0)
                                else:
                                    nc.gpsimd.tensor_add(y_t[h][:], y_t[h][:], sig[h][:])

                    for h in range(2):
                        nc.vector.tensor_copy(y16[h][:], y_t[h][:])
                        nc.sync.dma_start(y_loc[128 * h : 128 * (h + 1), :], y16[h][:])

                if comm:
                    nc.gpsimd.collective_compute(
                        "AllGather",
                        ALU.bypass,
                        replica_groups=GROUPS,
                        ins=[y_loc.opt()],
                        outs=[y_gat.opt()],
                    )
                else:  # timing-only stand-in, same data volume
                    for _g in range(4):
                        nc.sync.dma_start(
                            y_gat[CH * _g : CH * (_g + 1), :], y_loc[:, :]
                        )

                # ---------------- Phase B: channel mix + LayerNorm ----------------
                if not do_pb:
                    with tc.tile_pool(name="skip", bufs=1) as sk:
                        for o in range(2):
                            tt = sk.tile([128, L], F32, tag="sk")
                            nc.sync.dma_start(tt[:], xs[128 * o : 128 * (o + 1), :])
                            nc.sync.dma_start(og[128 * o : 128 * (o + 1), :], tt[:])
                    continue
                with (
                    tc.tile_pool(name="mix", bufs=1) as mx,
                    tc.tile_pool(name="yld", bufs=1) as yld,
                    tc.tile_pool(name="scr", bufs=2) as scr,
                    tc.tile_pool(name="tiny", bufs=2) as tiny,
                ):
                    wsb = mx.tile([128, 8 * CH], BF16, tag="wsb")      # lhsT per k-chunk
                    xsb = mx.tile([128, 2 * L], F32, tag="xsb")        # residual (o-major)
                    zsb = mx.tile([128, 2 * L], F32R, tag="zsb")
                    osb = mx.tile([128, 2 * L], F32, tag="osb")
                    bsc = smalls.tile([128, 2], F32, tag="bsc")
                    grow = smalls.tile([1, CH], F32R, tag="grow")
                    brow = smalls.tile([1, CH], F32R, tag="brow")
                    ones = smalls.tile([128, 1], F32R, tag="ones")
                    one_r = smalls.tile([1, NMM], F32R, tag="oner")
                    eps_t = smalls.tile([1, 1], F32, tag="eps")

                    for k in range(8):
                        nc.sync.dma_start(
                            wsb[:, CH * k : CH * (k + 1)], wTs[128 * k : 128 * (k + 1), :]
                        )
                    for o in range(2):
                        rs = slice(128 * o, 128 * (o + 1))
                        nc.sync.dma_start(xsb[:, L * o : L * (o + 1)], xs[rs, :])
                        nc.sync.dma_start(bsc[:, o : o + 1], bmixs[rs, :])

                    with tc.tile_pool(name="stage2", bufs=2) as stage2:
                        g32 = stage2.tile([1, CH], F32, tag="g32")
                        b32 = stage2.tile([1, CH], F32, tag="b32")
                        o32 = stage2.tile([128, 1], F32, tag="o32")
                        or32 = stage2.tile([1, NMM], F32, tag="or32")
                        nc.sync.dma_start(g32[:], gams[:])
                        nc.sync.dma_start(b32[:], bets[:])
                        nc.vector.tensor_copy(grow[:], g32[:])
                        nc.vector.tensor_copy(brow[:], b32[:])
                        nc.vector.memset(o32[:], 1.0)
                        nc.vector.tensor_copy(ones[:], o32[:])
                        nc.vector.memset(eps_t[:], LN_EPS)
                        nc.vector.memset(or32[:], 1.0)
                        nc.vector.tensor_copy(one_r[:], or32[:])

                    # matmul + residual + partial stats, halves of the position axis
                    with (
                        tc.tile_pool(name="mmps", bufs=4, space="PSUM") as psmm,
                        tc.tile_pool(name="stps", bufs=2, space="PSUM") as psst,
                    ):
                        for ph in range(2):
                            yhs = yld.tile([128, 8 * (L // 2)], BF16, tag="yhs")
                            for k in range(8):
                                nc.sync.dma_start(
                                    yhs[:, (L // 2) * k : (L // 2) * (k + 1)],
                                    y_gat[128 * k : 128 * (k + 1),
                                          (L // 2) * ph : (L // 2) * (ph + 1)],
                                )
                            for nth in range(L // 2 // NMM):   # 4 n-tiles per half
                                n0 = (L // 2) * ph + NMM * nth  # global position offset
                                pms = []
                                for o in range(2):
                                    pm = psmm.tile([128, NMM], F32, tag="mm")
                                    pms.append(pm)
                                    for k in range(8):
                                        nc.tensor.matmul(
                                            pm[:],
                                            wsb[:, CH * k + 128 * o :
                                                CH * k + 128 * (o + 1)],
                                            yhs[:, (L // 2) * k + NMM * nth :
                                                (L // 2) * k + NMM * (nth + 1)],
                                            start=(k == 0),
                                            stop=(k == 7),
                                        )
                                ps_sum = psst.tile([1, NMM], F32, tag="sum")
                                ps_sq = psst.tile([1, NMM], F32, tag="sq")
                                for o in range(2):
                                    zc = slice(L * o + n0, L * o + n0 + NMM)
                                    nc.vector.scalar_tensor_tensor(
                                        zsb[:, zc], pms[o][:], bsc[:, o : o + 1],
                                        xsb[:, zc], ALU.add, ALU.add,
                                    )
                                    nc.tensor.matmul(
                                        ps_sum[:], ones[:], zsb[:, zc],
                                        start=(o == 0), stop=(o == 1),
                                        skip_group_check=True,
                                    )
                                    z2 = scr.tile([128, NMM], F32R, tag="z2")
                                    nc.scalar.square(z2[:], zsb[:, zc])
                                    nc.tensor.matmul(
                                        ps_sq[:], ones[:], z2[:],
                                        start=(o == 0), stop=(o == 1),
                                        skip_group_check=True,
                                    )
                                sc_sum = tiny.tile([1, NMM], F32, tag="scsum")
                                sc_sq = tiny.tile([1, NMM], F32, tag="scsq")
                                nc.vector.tensor_copy(sc_sum[:], ps_sum[:])
                                nc.vector.tensor_copy(sc_sq[:], ps_sq[:])
                                nc.sync.dma_start(
                                    st_loc[0:1, n0 : n0 + NMM], sc_sum[:]
                                )
                                nc.sync.dma_start(
                                    st_loc[1:2, n0 : n0 + NMM], sc_sq[:]
                                )

                    if comm:
                        nc.gpsimd.collective_compute(
                            "AllReduce",
                            ALU.add,
                            replica_groups=GROUPS,
                            ins=[st_loc.opt()],
                            outs=[st_glb.opt()],
                        )
                    else:
                        nc.sync.dma_start(st_glb[:, :], st_loc[:, :])

                    # normalize per 512-position tile: out = z*G + B2
                    with tc.tile_pool(name="gbps", bufs=2, space="PSUM") as psgb:
                        for nt in range(L // NMM):
                            nn = slice(NMM * nt, NMM * (nt + 1))
                            mu = tiny.tile([1, NMM], F32R, tag="mu")
                            e2 = tiny.tile([1, NMM], F32, tag="e2")
                            m2 = tiny.tile([1, NMM], F32, tag="m2")
                            std = tiny.tile([1, NMM], F32, tag="std")
                            inv = tiny.tile([1, NMM], F32R, tag="inv")
                            nms = tiny.tile([1, NMM], F32R, tag="nms")
                            nc.sync.dma_start(m2[:], st_glb[0:1, nn])
                            nc.sync.dma_start(e2[:], st_glb[1:2, nn])
                            nc.vector.tensor_scalar_mul(mu[:], m2[:], 1.0 / D)
                            nc.vector.tensor_scalar_mul(e2[:], e2[:], 1.0 / D)
                            nc.vector.scalar_tensor_tensor(
                                m2[:], mu[:], -1.0, mu[:], ALU.mult, ALU.mult
                            )
                            nc.vector.tensor_add(m2[:], m2[:], e2[:])
                            nc.scalar.activation(std[:], m2[:], AF.Sqrt, bias=eps_t[:])
                            with nc.allow_low_precision(
                                reason="inv_std stored fp32r for PE outer-products"
                            ):
                                nc.vector.reciprocal(inv[:], std[:])
                            nc.vector.scalar_tensor_tensor(
                                nms[:], mu[:], -1.0, inv[:], ALU.mult, ALU.mult
                            )
                            for o in range(2):
                                oc = slice(128 * o, 128 * (o + 1))
                                zc = slice(L * o + NMM * nt, L * o + NMM * (nt + 1))
                                G = psgb.tile([128, NMM], F32, tag="G")
                                B2 = psgb.tile([128, NMM], F32, tag="B2")
                                nc.tensor.matmul(G[:], grow[:, oc], inv[:])
                                nc.tensor.matmul(
                                    B2[:], brow[:, oc], one_r[:],
                                    start=True, stop=False,
                                )
                                nc.tensor.matmul(
                                    B2[:], grow[:, oc], nms[:],
                                    start=False, stop=True,
                                )
                                nc.vector.scalar_tensor_tensor(
                                    osb[:, zc], zsb[:, zc], 1.0, G[:],
                                    ALU.mult, ALU.mult,
                                )
                                nc.vector.scalar_tensor_tensor(
                                    osb[:, zc], osb[:, zc], 1.0, B2[:],
                                    ALU.mult, ALU.add,
                                )
                    for o in range(2):
                        nc.sync.dma_start(
                            og[128 * o : 128 * (o + 1), :],
                            osb[:, L * o : L * (o + 1)],
                        )

    nc.compile()
    return nc


def _get_program(repeats: int = 1, comm: bool = True, do_tree: bool = True, do_pb: bool = True):
    key = f"nc{repeats}_{comm}_{do_tree}_{do_pb}"
    if key not in _CACHE:
        _CACHE[key] = _build_program(repeats, comm, do_tree, do_pb)
    return _CACHE[key]


def _make_in_maps(inputs):
    x = np.ascontiguousarray(np.asarray(inputs["x"], dtype=np.float32))
    h0 = np.asarray(inputs["h0"], dtype=np.float32)[:, 0, :]  # [D, FS]
    h1 = np.asarray(inputs["h1"], dtype=np.float32)[:, 0, :]
    w = np.asarray(inputs["w_mix"], dtype=np.float32)
    bm = np.asarray(inputs["b_mix"], dtype=np.float32).reshape(D, 1)
    gm = np.asarray(inputs["ln_gamma"], dtype=np.float32).reshape(1, D)
    bt = np.asarray(inputs["ln_beta"], dtype=np.float32).reshape(1, D)

    wT16 = np.ascontiguousarray(w.T).astype(ml_dtypes.bfloat16)  # [c, o]

    in_maps = []
    for c in range(NC):
        beta, gamma = c // 4, c % 4
        cs = slice(CH * gamma, CH * (gamma + 1))
        h0c = h0[cs].astype(ml_dtypes.bfloat16)
        h1c = h1[cs].astype(ml_dtypes.bfloat16)
        d0m = np.zeros((2, FS, 128, 128), ml_dtypes.bfloat16)
        d1m = np.zeros((2, FS, 128, 128), ml_dtypes.bfloat16)
        for h in range(2):
            for k in range(FS):
                np.fill_diagonal(d0m[h, k], h0c[128 * h : 128 * (h + 1), k])
                np.fill_diagonal(d1m[h, k], h1c[128 * h : 128 * (h + 1), k])
        in_maps.append(
            {
                "xs": np.ascontiguousarray(x[beta, cs, :]),
                "h0s": np.ascontiguousarray(h0[cs]),
                "h1s": np.ascontiguousarray(h1[cs]),
                "d0": d0m,
                "d1": d1m,
                "wTs": np.ascontiguousarray(wT16[:, cs]),
                "bmixs": np.ascontiguousarray(bm[cs]),
                "gams": np.ascontiguousarray(gm[:, cs]),
                "bets": np.ascontiguousarray(bt[:, cs]),
            }
        )
    return in_maps


def kernel(**inputs) -> np.ndarray:
    in_maps = _make_in_maps(inputs)
    nc = _get_program()
    res = run_bass_kernel_spmd(nc, in_maps, list(range(NC)))

    out = np.empty((B, D, L), dtype=np.float32)
    for c in range(NC):
        beta, gamma = c // 4, c % 4
        out[beta, CH * gamma : CH * (gamma + 1), :] = res.results[c]["og"]
    return out



# revision 6
# speedup vs baseline: 1.2949x; 1.2949x over previous
"""Trainium2 Bass kernel for nn_CustomMultiresLayer (B=2, D=1024, L=4096, FS=4).

Sharding (8 cores): core c -> batch beta=c//4, channel shard gamma=c%4
(256 channels). The multires tree + gated combination run position-chunked
(NCH chunks of CW columns) so the per-chunk AllGather of the gated tensor
y overlaps the next chunk's tree compute, and the channel-mix / LayerNorm
pipeline for chunk c overlaps the collectives of chunk c+1.

Engine plan (bf16 tree): conv taps are diagonal-weight matmuls on PE
(tap-outer to amortize LDWEIGHTS), ACT evacuates the a-chain + sigmoids,
DVE evacuates b-convs and does gating; a few b-levels run on DVE
directly. Causality makes chunking exact: each level keeps a small tail
of its output (full-width buffers for the two deepest levels) for the
next chunk's dilated convs. LayerNorm stats AllReduce per chunk
([2,CW] fp32), finish in a [128,CW/128] layout (128-lane reciprocal),
and are partition-broadcast for a PSUM-free DVE normalize.
"""

import numpy as np
import ml_dtypes

import concourse.bacc as bacc
import concourse.mybir as mybir
import concourse.tile as tile
from concourse.bass_utils import run_bass_kernel_spmd

F32 = mybir.dt.float32
BF16 = mybir.dt.bfloat16
AF = mybir.ActivationFunctionType
ALU = mybir.AluOpType

B, D, L = 2, 1024, 4096
FS, DEPTH = 4, 11
LN_EPS = 1e-5
NC = 8
CH = 256          # channels per core (2 half-tiles of 128)
NMM = 512         # PE moving-dim limit
NCH = 4           # position chunks
CW = L // NCH     # chunk width
NPP = CW // 128   # stats cols per partition
GROUPS = [[0, 1, 2, 3], [4, 5, 6, 7]]

# levels whose a-output must stay full-width (next level's shift exceeds CW)
FULL_A_LEVELS = frozenset(l for l in range(DEPTH - 1) if 3 * (1 << (l + 1)) > CW)
# b-conv levels computed on DVE instead of PE (level 0 always: doubled h1)
DVE_B_LEVELS = frozenset({0, 4, 6})

_CACHE = {}


def _tail_w(lvl):
    """Tail width kept from level `lvl`'s a-output for the next chunk."""
    return min(3 * (1 << (lvl + 1)), CW)


def _emit_conv_pe(nc, cps, evac, diag, dil, src=None, tail=None, tw=0,
                  src_full=None, cbase=0):
    """4-tap dilated causal depthwise conv for one chunk, tap-outer.
    Either src ([128,CW] chunk tile, + optional tail [128,tw]) or
    src_full ([128,L] tile read at global offset cbase)."""
    pp = cps.tile([128, CW], F32, tag="cps", name="cps")
    nblk = CW // NMM
    pieces = [[] for _ in range(nblk)]
    for k in (3, 2, 1, 0):
        s = (3 - k) * dil
        for bi in range(nblk):
            m0 = NMM * bi
            if src_full is not None:
                base = cbase + m0
                lo = max(0, s - base)
                if lo < NMM:
                    pieces[bi].append((k, s, "full", lo, NMM))
            else:
                if m0 < s and tail is not None:
                    hi = min(NMM, s - m0)
                    pieces[bi].append((k, s, "tail", 0, hi))
                lo = max(0, s - m0)
                if lo < NMM:
                    pieces[bi].append((k, s, "main", lo, NMM))
    for k in (3, 2, 1, 0):
        for bi in range(nblk):
            m0 = NMM * bi
            plist = pieces[bi]
            for pi, (pk, s, kind, lo, hi) in enumerate(plist):
                if pk != k:
                    continue
                if kind == "full":
                    rhs = src_full[:, cbase + m0 + lo - s : cbase + m0 + NMM - s]
                elif kind == "main":
                    rhs = src[:, m0 + lo - s : m0 + NMM - s]
                else:
                    rhs = tail[:, tw - s + m0 : tw - s + m0 + hi]
                nc.tensor.matmul(
                    pp[:, m0 + lo : m0 + hi],
                    diag[:, 128 * k : 128 * (k + 1)],
                    rhs,
                    start=(pi == 0),
                    stop=(pi == len(plist) - 1),
                )
    evac(pp)


def _emit_conv_dve(nc, dst, h, dil, src=None, tail=None, tw=0,
                   src_full=None, cbase=0, tmp=None):
    """4-tap conv on DVE for one chunk: tensor_scalar tap-3, then either
    tensor_scalar+shifted-add pairs (even s, 2x mode) or STT."""
    if src_full is not None:
        nc.vector.tensor_scalar_mul(dst[:], src_full[:, cbase : cbase + CW], h[:, 3:4])
    else:
        nc.vector.tensor_scalar_mul(dst[:], src[:], h[:, 3:4])
    for k in (2, 1, 0):
        s = (3 - k) * dil
        if src_full is not None:
            lo = max(0, s - cbase)
            if lo < CW:
                nc.vector.scalar_tensor_tensor(
                    dst[:, lo:CW],
                    src_full[:, cbase + lo - s : cbase + CW - s],
                    h[:, k : k + 1], dst[:, lo:CW], ALU.mult, ALU.add,
                )
            continue
        if s < CW and tmp is not None and s % 2 == 0:
            nc.vector.tensor_scalar_mul(tmp[:, 0 : CW - s], src[:, 0 : CW - s],
                                        h[:, k : k + 1])
            nc.vector.tensor_add(dst[:, s:CW], dst[:, s:CW], tmp[:, 0 : CW - s])
        elif s < CW:
            nc.vector.scalar_tensor_tensor(
                dst[:, s:CW], src[:, 0 : CW - s], h[:, k : k + 1],
                dst[:, s:CW], ALU.mult, ALU.add,
            )
        if tail is not None and s > 0:
            nc.vector.scalar_tensor_tensor(
                dst[:, 0 : min(s, CW)], tail[:, tw - s : tw - s + min(s, CW)],
                h[:, k : k + 1], dst[:, 0 : min(s, CW)], ALU.mult, ALU.add,
            )


def _build_program():
    nc = bacc.Bacc("TRN2", target_bir_lowering=False, debug=False, num_devices=NC)

    xs16 = nc.dram_tensor("xs16", [CH, L], BF16, kind="ExternalInput").ap()
    d0 = nc.dram_tensor("d0", [2, FS, 128, 128], BF16, kind="ExternalInput").ap()
    d1 = nc.dram_tensor("d1", [2, FS, 128, 128], BF16, kind="ExternalInput").ap()
    h1x2 = nc.dram_tensor("h1x2", [CH, FS], F32, kind="ExternalInput").ap()
    h1n = nc.dram_tensor("h1n", [CH, FS], F32, kind="ExternalInput").ap()
    wTs = nc.dram_tensor("wTs", [D, CH], BF16, kind="ExternalInput").ap()
    bmixs = nc.dram_tensor("bmixs", [CH, 1], F32, kind="ExternalInput").ap()
    gamc = nc.dram_tensor("gamc", [CH, 1], F32, kind="ExternalInput").ap()
    betc = nc.dram_tensor("betc", [CH, 1], F32, kind="ExternalInput").ap()
    og = nc.dram_tensor("og", [CH, L], F32, kind="ExternalOutput").ap()

    with tile.TileContext(nc) as tc:
        with (
            tc.tile_pool(name="dram", bufs=1, space="DRAM") as dram,
            tc.tile_pool(name="smalls", bufs=1) as smalls,
            tc.tile_pool(name="tree", bufs=1) as tp,
            tc.tile_pool(name="cpsum", bufs=2, space="PSUM") as cps,
            tc.tile_pool(name="mix", bufs=1) as mx,
            tc.tile_pool(name="scr", bufs=2) as scr,
            tc.tile_pool(name="tiny", bufs=1) as tiny,
            tc.tile_pool(name="mmps", bufs=2, space="PSUM") as psmm,
            tc.tile_pool(name="stps", bufs=1, space="PSUM") as psst,
        ):
            y_loc = [dram.tile([CH, CW], BF16, name=f"yloc{c}") for c in range(NCH)]
            y_gat = [dram.tile([D, CW], BF16, name=f"ygat{c}") for c in range(NCH)]
            st_loc = [dram.tile([2, CW], F32, name=f"stloc{c}") for c in range(NCH)]
            st_glb = [dram.tile([2, CW], F32, name=f"stglb{c}") for c in range(NCH)]

            # ---- persistent loads ----
            wsb = smalls.tile([128, 8 * CH], BF16, tag="wsb", name="wsb")
            xf = [smalls.tile([128, L], BF16, tag=f"xf{h}", name=f"xf{h}") for h in range(2)]
            h2c = [smalls.tile([128, FS], F32, tag=f"h2c{h}", name=f"h2c{h}") for h in range(2)]
            h1c = [smalls.tile([128, FS], F32, tag=f"h1c{h}", name=f"h1c{h}") for h in range(2)]
            d0c = [smalls.tile([128, FS * 128], BF16, tag=f"d0c{h}", name=f"d0c{h}") for h in range(2)]
            d1c = [smalls.tile([128, FS * 128], BF16, tag=f"d1c{h}", name=f"d1c{h}") for h in range(2)]
            bsc = smalls.tile([128, 2], F32, tag="bsc", name="bsc")
            gsc = smalls.tile([128, 2], F32, tag="gsc", name="gsc")
            btc = smalls.tile([128, 2], F32, tag="btc", name="btc")
            ones16 = smalls.tile([128, 1], BF16, tag="ones16", name="ones16")
            eps_t = smalls.tile([128, 1], F32, tag="eps", name="eps")

            for k in range(8):
                nc.sync.dma_start(wsb[:, CH * k : CH * (k + 1)],
                                  wTs[128 * k : 128 * (k + 1), :])
            for h in range(2):
                rs = slice(128 * h, 128 * (h + 1))
                nc.sync.dma_start(xf[h][:], xs16[rs, :])
                nc.sync.dma_start(h2c[h][:], h1x2[rs, :])
                nc.sync.dma_start(h1c[h][:], h1n[rs, :])
                for k in range(FS):
                    ks = slice(128 * k, 128 * (k + 1))
                    nc.sync.dma_start(d0c[h][:, ks], d0[h, k])
                    nc.sync.dma_start(d1c[h][:, ks], d1[h, k])
                nc.sync.dma_start(bsc[:, h : h + 1], bmixs[rs, :])
                nc.sync.dma_start(gsc[:, h : h + 1], gamc[rs, :])
                nc.sync.dma_start(btc[:, h : h + 1], betc[rs, :])
            with tc.tile_pool(name="stage0", bufs=1) as st0:
                o32 = st0.tile([128, 1], F32, tag="o32", name="o32")
                nc.vector.memset(o32[:], 1.0)
                nc.vector.tensor_copy(ones16[:], o32[:])
                nc.vector.memset(eps_t[:], LN_EPS)

            # ---- tree tiles ----
            a_t = [[tp.tile([128, CW], BF16, tag=f"a{h}{i}", name=f"a{h}{i}")
                    for i in range(2)] for h in range(2)]
            af = {(h, l): tp.tile([128, L], BF16, tag=f"af{h}{l}", name=f"af{h}{l}")
                  for h in range(2) for l in FULL_A_LEVELS}
            ta = {(h, l): [tp.tile([128, _tail_w(l)], BF16, tag=f"ta{h}{l}{p}",
                                   name=f"ta{h}{l}{p}") for p in range(2)]
                  for h in range(2) for l in range(DEPTH - 1) if l not in FULL_A_LEVELS}
            b_t = [[tp.tile([128, CW], BF16, tag=f"b{h}{i}", name=f"b{h}{i}")
                    for i in range(2)] for h in range(2)]
            sig = [tp.tile([128, CW], BF16, tag=f"s{h}", name=f"s{h}") for h in range(2)]
            g_t = [tp.tile([128, CW], BF16, tag=f"g{h}", name=f"g{h}") for h in range(2)]
            y_t = [[tp.tile([128, CW], BF16, tag=f"y{h}{p}", name=f"y{h}{p}")
                    for p in range(2)] for h in range(2)]
            tmp = tp.tile([128, CW], BF16, tag="tmp", name="tmp")

            # ---- phase-B tiles ----
            yhs = [mx.tile([128, CW], BF16, tag=f"yh{k}", name=f"yh{k}") for k in range(8)]
            zsb = [mx.tile([128, 2 * CW], BF16, tag=f"zsb{p}", name=f"zsb{p}")
                   for p in range(2)]
            invb = [mx.tile([128, CW], F32, tag=f"invb{p}", name=f"invb{p}")
                    for p in range(2)]
            nmsb = [mx.tile([128, CW], F32, tag=f"nmsb{p}", name=f"nmsb{p}")
                    for p in range(2)]  # fp32: STT runs 1x regardless

            def emit_phase_b(c):
                zs = zsb[c % 2]
                for k in range(8):
                    nc.sync.dma_start(yhs[k][:], y_gat[c][128 * k : 128 * (k + 1), :])
                for o in range(2):
                    zc = slice(CW * o, CW * (o + 1))
                    for bi in range(CW // NMM):
                        bs = slice(NMM * bi, NMM * (bi + 1))
                        zbs = slice(CW * o + NMM * bi, CW * o + NMM * (bi + 1))
                        pm = psmm.tile([128, NMM], F32, tag="pm", name="pm")
                        for k in range(8):
                            nc.tensor.matmul(
                                pm[:],
                                wsb[:, CH * k + 128 * o : CH * k + 128 * (o + 1)],
                                yhs[k][:, bs],
                                start=(k == 0), stop=(k == 7),
                            )
                        nc.vector.scalar_tensor_tensor(
                            zs[:, zbs], pm[:], bsc[:, o : o + 1],
                            xf[o][:, CW * c + NMM * bi : CW * c + NMM * (bi + 1)],
                            ALU.add, ALU.add,
                        )
                sc_sum = tiny.tile([1, CW], F32, tag="scs", name="scs")
                sc_sq = tiny.tile([1, CW], F32, tag="scq", name="scq")
                for bi in range(CW // NMM):
                    bs = slice(NMM * bi, NMM * (bi + 1))
                    ps_sum = psst.tile([1, NMM], F32, tag="sts", name="sts")
                    ps_sq = psst.tile([1, NMM], F32, tag="stq", name="stq")
                    for o in range(2):
                        zbs = slice(CW * o + NMM * bi, CW * o + NMM * (bi + 1))
                        nc.tensor.matmul(
                            ps_sum[:], ones16[:], zs[:, zbs],
                            start=(o == 0), stop=(o == 1), skip_group_check=True,
                        )
                        z2 = scr.tile([128, NMM], BF16, tag="z2", name="z2")
                        nc.scalar.square(z2[:], zs[:, zbs])
                        nc.tensor.matmul(
                            ps_sq[:], ones16[:], z2[:],
                            start=(o == 0), stop=(o == 1), skip_group_check=True,
                        )
                    nc.scalar.copy(sc_sum[:, bs], ps_sum[:])
                    nc.scalar.copy(sc_sq[:, bs], ps_sq[:])
                nc.gpsimd.dma_start(st_loc[c][0:1, :], sc_sum[:])
                nc.gpsimd.dma_start(st_loc[c][1:2, :], sc_sq[:])
                nc.gpsimd.collective_compute(
                    "AllReduce", ALU.add, replica_groups=GROUPS,
                    ins=[st_loc[c].opt()], outs=[st_glb[c].opt()],
                )
                # stats finish in [128, NPP] layout (t = NPP*p + j)
                stt = tiny.tile([128, 2 * NPP], F32, tag="stt", name="stt")
                mu = tiny.tile([128, NPP], F32, tag="mu", name="mu")
                e2 = tiny.tile([128, NPP], F32, tag="e2", name="e2")
                m2 = tiny.tile([128, NPP], F32, tag="m2", name="m2")
                std = tiny.tile([128, NPP], F32, tag="std", name="std")
                inv = tiny.tile([128, NPP], F32, tag="inv", name="inv")
                nms = tiny.tile([128, NPP], F32, tag="nms", name="nms")
                ivr = tiny.tile([1, CW], F32, tag="ivr", name="ivr")
                nmr = tiny.tile([1, CW], F32, tag="nmr", name="nmr")
                nc.gpsimd.dma_start(stt[:, 0:NPP], st_glb[c][0:1, :])
                nc.gpsimd.dma_start(stt[:, NPP : 2 * NPP], st_glb[c][1:2, :])
                nc.vector.tensor_scalar_mul(mu[:], stt[:, 0:NPP], 1.0 / D)
                nc.vector.tensor_scalar_mul(e2[:], stt[:, NPP : 2 * NPP], 1.0 / D)
                nc.vector.tensor_mul(m2[:], mu[:], mu[:])
                nc.vector.tensor_sub(e2[:], e2[:], m2[:])
                nc.scalar.activation(std[:], e2[:], AF.Sqrt, bias=eps_t[:])
                with nc.allow_low_precision(reason="LN inv_std"):
                    nc.vector.reciprocal(inv[:], std[:])
                nc.vector.scalar_tensor_tensor(nms[:], mu[:], -1.0, inv[:],
                                               ALU.mult, ALU.mult)
                nc.gpsimd.dma_start(ivr[:], inv[:])
                nc.gpsimd.dma_start(nmr[:], nms[:])
                ib, nb = invb[c % 2], nmsb[c % 2]
                nc.gpsimd.partition_broadcast(ib[:], ivr[:])
                nc.gpsimd.partition_broadcast(nb[:], nmr[:])
                # out = (z*gamma)*inv + (nms*gamma) + beta
                for o in range(2):
                    zc = slice(CW * o, CW * (o + 1))
                    t1 = scr.tile([128, CW], F32, tag="t1", name="t1")
                    t2 = scr.tile([128, CW], F32, tag="t2", name="t2")
                    ost = scr.tile([128, CW], F32, tag="ost", name="ost")
                    nc.vector.scalar_tensor_tensor(
                        t1[:], zs[:, zc], gsc[:, o : o + 1], ib[:], ALU.mult, ALU.mult)
                    nc.vector.scalar_tensor_tensor(
                        t2[:], nb[:], gsc[:, o : o + 1], t1[:], ALU.mult, ALU.add)
                    nc.scalar.activation(ost[:], t2[:], AF.Identity,
                                         bias=btc[:, o : o + 1])
                    nc.sync.dma_start(
                        og[128 * o : 128 * (o + 1), CW * c : CW * (c + 1)], ost[:])

            # ================= main chunk loop =================
            for c in range(NCH):
                cbase = CW * c
                for lvl in range(DEPTH):
                    dil = 1 << lvl
                    for h in range(2):
                        # --- resolve input of this level ---
                        if lvl == 0:
                            src, tail, tw, src_full = None, None, 0, xf[h]
                        elif (lvl - 1) in FULL_A_LEVELS:
                            src, tail, tw, src_full = None, None, 0, af[(h, lvl - 1)]
                        else:
                            src = a_t[h][(lvl - 1) % 2]
                            tw = _tail_w(lvl - 1)
                            tail = ta[(h, lvl - 1)][(c + 1) % 2] if c > 0 else None
                            src_full = None
                        # --- output storage of this level's a-conv ---
                        if lvl in FULL_A_LEVELS:
                            a_out = af[(h, lvl)]
                            a_dst = a_out[:, cbase : cbase + CW]
                        else:
                            a_out = a_t[h][lvl % 2]
                            a_dst = a_out[:, 0:CW]

                        def evac_a(pp, h=h, lvl=lvl, c=c, a_out=a_out, a_dst=a_dst):
                            nc.scalar.copy(a_dst, pp[:])
                            if lvl >= 1:
                                nc.scalar.activation(sig[h][:], pp[:], AF.Sigmoid)
                            if (lvl < DEPTH - 1 and lvl not in FULL_A_LEVELS
                                    and c < NCH - 1):
                                w = _tail_w(lvl)
                                nc.scalar.copy(ta[(h, lvl)][c % 2][:],
                                               a_out[:, CW - w : CW])

                        _emit_conv_pe(nc, cps, evac_a, d0c[h], dil,
                                      src=src, tail=tail, tw=tw,
                                      src_full=src_full, cbase=cbase)

                        if lvl >= 1:
                            b_prv = b_t[h][(lvl - 1) % 2]
                            nc.vector.tensor_mul(g_t[h][:], sig[h][:], b_prv[:])
                            yt = y_t[h][c % 2]
                            if lvl == 1:
                                nc.vector.tensor_copy(yt[:], g_t[h][:])
                            else:
                                nc.vector.tensor_add(yt[:], yt[:], g_t[h][:])

                        if lvl < DEPTH - 1:
                            b_cur = b_t[h][lvl % 2]
                            if lvl in DVE_B_LEVELS:
                                hh = h2c[h] if lvl == 0 else h1c[h]
                                _emit_conv_dve(nc, b_cur, hh, dil,
                                               src=src, tail=tail, tw=tw,
                                               src_full=src_full, cbase=cbase,
                                               tmp=tmp)
                            else:
                                def evac_b(pp, b_cur=b_cur):
                                    nc.vector.tensor_copy(b_cur[:], pp[:])
                                _emit_conv_pe(nc, cps, evac_b, d1c[h], dil,
                                              src=src, tail=tail, tw=tw,
                                              src_full=src_full, cbase=cbase)

                for h in range(2):
                    nc.sync.dma_start(y_loc[c][128 * h : 128 * (h + 1), :],
                                      y_t[h][c % 2][:])
                nc.gpsimd.collective_compute(
                    "AllGather", ALU.bypass, replica_groups=GROUPS,
                    ins=[y_loc[c].opt()], outs=[y_gat[c].opt()],
                )
                if c >= 1:
                    emit_phase_b(c - 1)
            emit_phase_b(NCH - 1)

    nc.compile()
    return nc


def _get_program():
    if "nc" not in _CACHE:
        _CACHE["nc"] = _build_program()
    return _CACHE["nc"]


def _make_in_maps(inputs):
    x = np.ascontiguousarray(np.asarray(inputs["x"], dtype=np.float32))
    h0 = np.asarray(inputs["h0"], dtype=np.float32)[:, 0, :]  # [D, FS]
    h1 = np.asarray(inputs["h1"], dtype=np.float32)[:, 0, :]
    w = np.asarray(inputs["w_mix"], dtype=np.float32)
    bm = np.asarray(inputs["b_mix"], dtype=np.float32).reshape(D, 1)
    gm = np.asarray(inputs["ln_gamma"], dtype=np.float32).reshape(D, 1)
    bt = np.asarray(inputs["ln_beta"], dtype=np.float32).reshape(D, 1)

    wT16 = np.ascontiguousarray(w.T).astype(ml_dtypes.bfloat16)  # [c, o]

    in_maps = []
    for c in range(NC):
        beta, gamma = c // 4, c % 4
        cs = slice(CH * gamma, CH * (gamma + 1))
        h0c = h0[cs].astype(ml_dtypes.bfloat16)
        h1c = h1[cs].astype(ml_dtypes.bfloat16)
        d0m = np.zeros((2, FS, 128, 128), ml_dtypes.bfloat16)
        d1m = np.zeros((2, FS, 128, 128), ml_dtypes.bfloat16)
        for h in range(2):
            for k in range(FS):
                np.fill_diagonal(d0m[h, k], h0c[128 * h : 128 * (h + 1), k])
                np.fill_diagonal(d1m[h, k], h1c[128 * h : 128 * (h + 1), k])
        in_maps.append(
            {
                "xs16": np.ascontiguousarray(x[beta, cs, :]).astype(ml_dtypes.bfloat16),
                "d0": d0m,
                "d1": d1m,
                "h1x2": np.ascontiguousarray(2.0 * h1[cs]),
                "h1n": np.ascontiguousarray(h1[cs]),
                "wTs": np.ascontiguousarray(wT16[:, cs]),
                "bmixs": np.ascontiguousarray(bm[cs]),
                "gamc": np.ascontiguousarray(gm[cs]),
                "betc": np.ascontiguousarray(bt[cs]),
            }
        )
    return in_maps


def kernel(**inputs) -> np.ndarray:
    in_maps = _make_in_maps(inputs)
    nc = _get_program()
    res = run_bass_kernel_spmd(nc, in_maps, list(range(NC)))

    out = np.empty((B, D, L), dtype=np.float32)
    for c in range(NC):
        beta, gamma = c // 4, c % 4
        out[beta, CH * gamma : CH * (gamma + 1), :] = res.results[c]["og"]
    return out


# revision 9
# speedup vs baseline: 1.9773x; 1.5269x over previous
"""Trainium2 Bass kernel for nn_CustomMultiresLayer (B=2, D=1024, L=4096, FS=4).

Sharding (8 cores): core c -> batch beta=c//4, channel shard gamma=c%4
(256 channels). The multires tree + gated combination run position-chunked
(NCH=2 chunks of CW columns): the first chunk's AllGather of the gated
tensor y overlaps the second chunk's tree compute, and the channel-mix /
LayerNorm stats of chunk 0 overlap chunk 1's AllGather. A single bf16
AllReduce carries both chunks' LayerNorm partial sums (collective count
kept at 3 -- more collectives inflate the NEFF's startup barrier).

Engine plan (bf16 tree): conv taps are diagonal-weight matmuls on PE,
ACT evacuates the a-chain + sigmoids + PE-b-convs, DVE computes the
gating and roughly half the b-convs directly (tensor_scalar +
shifted-add pairs). Causality makes chunking exact: each level keeps a
tail of its output (full-width buffer for the deepest level) for the
next chunk's dilated convs. LayerNorm stats finish in a [128,CW/128]
layout (128-lane reciprocal) and are partition-broadcast for a
PSUM-free DVE normalize. Emission is split early/late so no engine
queue blocks tree work behind collective-dependent ops.
"""

import numpy as np
import ml_dtypes

import concourse.bacc as bacc
import concourse.mybir as mybir
import concourse.tile as tile
from concourse.bass_utils import run_bass_kernel_spmd

F32 = mybir.dt.float32
BF16 = mybir.dt.bfloat16
AF = mybir.ActivationFunctionType
ALU = mybir.AluOpType

B, D, L = 2, 1024, 4096
FS, DEPTH = 4, 11
LN_EPS = 1e-5
NC = 8
CH = 256          # channels per core (2 half-tiles of 128)
NMM = 512         # PE moving-dim limit
NCH = 2           # position chunks
CW = L // NCH     # chunk width
NPP = CW // 128   # stats cols per partition
NB = CW // 1024   # 1024-col blocks per chunk (norm granularity)
GROUPS = [[0, 1, 2, 3], [4, 5, 6, 7]]

# levels whose a-output must stay full-width (next level's shift exceeds CW)
FULL_A_LEVELS = frozenset(l for l in range(DEPTH - 1) if 3 * (1 << (l + 1)) > CW)
# b-conv levels computed on DVE instead of PE (level 0 always: doubled h1)
DVE_B_LEVELS = frozenset({0, 2, 3, 4, 6, 8, 9})

_CACHE = {}


def _tail_w(lvl):
    """Tail width kept from level `lvl`'s a-output for the next chunk."""
    return min(3 * (1 << (lvl + 1)), CW)


def _emit_conv_pe(nc, cps, evac, diag, dil, src=None, tail=None, tw=0,
                  src_full=None, cbase=0):
    """4-tap dilated causal depthwise conv for one chunk, tap-outer per
    1024-col PSUM tile. Either src ([128,CW] chunk tile, + optional tail
    [128,tw]) or src_full ([128,L] tile read at global offset cbase).
    evac(pp, cs) evacuates one PSUM tile covering chunk-local slice cs."""
    PW = 1024
    nsub = PW // NMM
    for p0 in range(0, CW, PW):
        pp = cps.tile([128, PW], F32, tag="cps", name="cps")
        pieces = [[] for _ in range(nsub)]
        for k in (3, 2, 1, 0):
            s = (3 - k) * dil
            for bi in range(nsub):
                m0 = p0 + NMM * bi          # chunk-local block offset
                if src_full is not None:
                    base = cbase + m0
                    lo = max(0, s - base)
                    if lo < NMM:
                        pieces[bi].append((k, s, "full", lo, NMM))
                else:
                    if m0 < s and tail is not None:
                        hi = min(NMM, s - m0)
                        pieces[bi].append((k, s, "tail", 0, hi))
                    lo = max(0, s - m0)
                    if lo < NMM:
                        pieces[bi].append((k, s, "main", lo, NMM))
        for k in (3, 2, 1, 0):
            for bi in range(nsub):
                m0 = p0 + NMM * bi
                po = NMM * bi               # offset within pp
                plist = pieces[bi]
                for pi, (pk, s, kind, lo, hi) in enumerate(plist):
                    if pk != k:
                        continue
                    if kind == "full":
                        rhs = src_full[:, cbase + m0 + lo - s : cbase + m0 + NMM - s]
                    elif kind == "main":
                        rhs = src[:, m0 + lo - s : m0 + NMM - s]
                    else:
                        rhs = tail[:, tw - s + m0 : tw - s + m0 + hi]
                    nc.tensor.matmul(
                        pp[:, po + lo : po + hi],
                        diag[:, 128 * k : 128 * (k + 1)],
                        rhs,
                        start=(pi == 0),
                        stop=(pi == len(plist) - 1),
                    )
        evac(pp, slice(p0, p0 + PW))


def _emit_conv_dve(nc, dst, h, dil, src=None, tail=None, tw=0,
                   src_full=None, cbase=0, tmp=None):
    """4-tap conv on DVE for one chunk: tensor_scalar tap-3, then either
    tensor_scalar+shifted-add pairs (even s, 2x mode) or STT."""
    if src_full is not None:
        nc.vector.tensor_scalar_mul(dst[:], src_full[:, cbase : cbase + CW], h[:, 3:4])
    else:
        nc.vector.tensor_scalar_mul(dst[:], src[:], h[:, 3:4])
    for k in (2, 1, 0):
        s = (3 - k) * dil
        if src_full is not None:
            lo = max(0, s - cbase)
            if lo < CW:
                nc.vector.scalar_tensor_tensor(
                    dst[:, lo:CW],
                    src_full[:, cbase + lo - s : cbase + CW - s],
                    h[:, k : k + 1], dst[:, lo:CW], ALU.mult, ALU.add,
                )
            continue
        if s < CW and tmp is not None and s % 2 == 0:
            nc.vector.tensor_scalar_mul(tmp[:, 0 : CW - s], src[:, 0 : CW - s],
                                        h[:, k : k + 1])
            nc.vector.tensor_add(dst[:, s:CW], dst[:, s:CW], tmp[:, 0 : CW - s])
        elif s < CW:
            nc.vector.scalar_tensor_tensor(
                dst[:, s:CW], src[:, 0 : CW - s], h[:, k : k + 1],
                dst[:, s:CW], ALU.mult, ALU.add,
            )
        if tail is not None and s > 0:
            nc.vector.scalar_tensor_tensor(
                dst[:, 0 : min(s, CW)], tail[:, tw - s : tw - s + min(s, CW)],
                h[:, k : k + 1], dst[:, 0 : min(s, CW)], ALU.mult, ALU.add,
            )


def _build_program():
    nc = bacc.Bacc("TRN2", target_bir_lowering=False, debug=False, num_devices=NC)

    xs16 = nc.dram_tensor("xs16", [CH, L], BF16, kind="ExternalInput").ap()
    d0 = nc.dram_tensor("d0", [2, FS, 128, 128], BF16, kind="ExternalInput").ap()
    d1 = nc.dram_tensor("d1", [2, FS, 128, 128], BF16, kind="ExternalInput").ap()
    h1x2 = nc.dram_tensor("h1x2", [CH, FS], F32, kind="ExternalInput").ap()
    h1n = nc.dram_tensor("h1n", [CH, FS], F32, kind="ExternalInput").ap()
    wTs = nc.dram_tensor("wTs", [D, CH], BF16, kind="ExternalInput").ap()
    bmixs = nc.dram_tensor("bmixs", [CH, 1], F32, kind="ExternalInput").ap()
    gamc = nc.dram_tensor("gamc", [CH, 1], F32, kind="ExternalInput").ap()
    betc = nc.dram_tensor("betc", [CH, 1], F32, kind="ExternalInput").ap()
    og = nc.dram_tensor("og", [CH, L], F32, kind="ExternalOutput").ap()

    with tile.TileContext(nc) as tc:
        with (
            tc.tile_pool(name="dram", bufs=1, space="DRAM") as dram,
            tc.tile_pool(name="smalls", bufs=1) as smalls,
            tc.tile_pool(name="tree", bufs=1) as tp,
            tc.tile_pool(name="cpsum", bufs=2, space="PSUM") as cps,
            tc.tile_pool(name="mix", bufs=1) as mx,
            tc.tile_pool(name="scr", bufs=2) as scr,
            tc.tile_pool(name="tiny", bufs=1) as tiny,
            tc.tile_pool(name="mmps", bufs=2, space="PSUM") as psmm,
            tc.tile_pool(name="stps", bufs=1, space="PSUM") as psst,
        ):
            y_loc = [dram.tile([CH, CW], BF16, name=f"yloc{c}") for c in range(NCH)]
            y_gat = [dram.tile([D, CW], BF16, name=f"ygat{c}") for c in range(NCH)]
            st_loc = dram.tile([2, L], BF16, name="stloc")
            st_glb = dram.tile([2, L], BF16, name="stglb")

            # ---- persistent loads ----
            wsb = smalls.tile([128, 8 * CH], BF16, tag="wsb", name="wsb")
            xf = [smalls.tile([128, L], BF16, tag=f"xf{h}", name=f"xf{h}") for h in range(2)]
            h2c = [smalls.tile([128, FS], F32, tag=f"h2c{h}", name=f"h2c{h}") for h in range(2)]
            h1c = [smalls.tile([128, FS], F32, tag=f"h1c{h}", name=f"h1c{h}") for h in range(2)]
            d0c = [smalls.tile([128, FS * 128], BF16, tag=f"d0c{h}", name=f"d0c{h}") for h in range(2)]
            d1c = [smalls.tile([128, FS * 128], BF16, tag=f"d1c{h}", name=f"d1c{h}") for h in range(2)]
            bsc = smalls.tile([128, 2], F32, tag="bsc", name="bsc")
            gsc = smalls.tile([128, 2], F32, tag="gsc", name="gsc")
            btc = smalls.tile([128, 2], F32, tag="btc", name="btc")
            ones16 = smalls.tile([128, 1], BF16, tag="ones16", name="ones16")
            eps_t = smalls.tile([128, 1], F32, tag="eps", name="eps")

            for k in range(8):
                nc.sync.dma_start(wsb[:, CH * k : CH * (k + 1)],
                                  wTs[128 * k : 128 * (k + 1), :])
            for h in range(2):
                rs = slice(128 * h, 128 * (h + 1))
                nc.sync.dma_start(xf[h][:], xs16[rs, :])
                nc.sync.dma_start(h2c[h][:], h1x2[rs, :])
                nc.sync.dma_start(h1c[h][:], h1n[rs, :])
                for k in range(FS):
                    ks = slice(128 * k, 128 * (k + 1))
                    nc.sync.dma_start(d0c[h][:, ks], d0[h, k])
                    nc.sync.dma_start(d1c[h][:, ks], d1[h, k])
                nc.sync.dma_start(bsc[:, h : h + 1], bmixs[rs, :])
                nc.sync.dma_start(gsc[:, h : h + 1], gamc[rs, :])
                nc.sync.dma_start(btc[:, h : h + 1], betc[rs, :])
            with tc.tile_pool(name="stage0", bufs=1) as st0:
                o32 = st0.tile([128, 1], F32, tag="o32", name="o32")
                nc.vector.memset(o32[:], 1.0)
                nc.vector.tensor_copy(ones16[:], o32[:])
                nc.vector.memset(eps_t[:], LN_EPS)

            # ---- tree tiles ----
            a_t = [[tp.tile([128, CW], BF16, tag=f"a{h}{i}", name=f"a{h}{i}")
                    for i in range(2)] for h in range(2)]
            af = {(h, l): tp.tile([128, L], BF16, tag=f"af{h}{l}", name=f"af{h}{l}")
                  for h in range(2) for l in FULL_A_LEVELS}
            ta = {(h, l): tp.tile([128, _tail_w(l)], BF16, tag=f"ta{h}{l}",
                                  name=f"ta{h}{l}")
                  for h in range(2) for l in range(DEPTH - 1) if l not in FULL_A_LEVELS}
            b_t = [[tp.tile([128, CW], BF16, tag=f"b{h}{i}", name=f"b{h}{i}")
                    for i in range(2)] for h in range(2)]
            sig = [tp.tile([128, CW], BF16, tag=f"s{h}", name=f"s{h}") for h in range(2)]
            g_t = [tp.tile([128, CW], BF16, tag=f"g{h}", name=f"g{h}") for h in range(2)]
            y_t = [tp.tile([128, CW], BF16, tag=f"y{h}", name=f"y{h}") for h in range(2)]
            tmp = tp.tile([128, CW], BF16, tag="tmp", name="tmp")

            # ---- phase-B tiles ----
            yhs = [mx.tile([128, CW], BF16, tag=f"yh{k}", name=f"yh{k}") for k in range(8)]
            zsb = [mx.tile([128, 2 * CW], BF16, tag=f"zsb{p}", name=f"zsb{p}")
                   for p in range(2)]
            invb = mx.tile([128, CW], BF16, tag="invb", name="invb")
            nmsb = mx.tile([128, CW], BF16, tag="nmsb", name="nmsb")

            def emit_pb_early(c):
                """mix + z + LN partial sums for chunk c (needs AG_c only)."""
                zs = zsb[c % 2]
                for k in range(8):
                    nc.sync.dma_start(yhs[k][:], y_gat[c][128 * k : 128 * (k + 1), :])
                sc_sum = tiny.tile([1, CW], BF16, tag="scs", name="scs")
                sc_sq = tiny.tile([1, CW], BF16, tag="scq", name="scq")
                for o in range(2):
                    for bi in range(CW // NMM):
                        bs = slice(NMM * bi, NMM * (bi + 1))
                        zbs = slice(CW * o + NMM * bi, CW * o + NMM * (bi + 1))
                        pm = psmm.tile([128, NMM], F32, tag="pm", name="pm")
                        for k in range(8):
                            nc.tensor.matmul(
                                pm[:],
                                wsb[:, CH * k + 128 * o : CH * k + 128 * (o + 1)],
                                yhs[k][:, bs],
                                start=(k == 0), stop=(k == 7),
                            )
                        nc.vector.scalar_tensor_tensor(
                            zs[:, zbs], pm[:], bsc[:, o : o + 1],
                            xf[o][:, CW * c + NMM * bi : CW * c + NMM * (bi + 1)],
                            ALU.add, ALU.add,
                        )
                for bi in range(CW // NMM):
                    bs = slice(NMM * bi, NMM * (bi + 1))
                    ps_sum = psst.tile([1, NMM], F32, tag="sts", name="sts")
                    ps_sq = psst.tile([1, NMM], F32, tag="stq", name="stq")
                    for o in range(2):
                        zbs = slice(CW * o + NMM * bi, CW * o + NMM * (bi + 1))
                        nc.tensor.matmul(
                            ps_sum[:], ones16[:], zs[:, zbs],
                            start=(o == 0), stop=(o == 1), skip_group_check=True,
                        )
                        z2 = scr.tile([128, NMM], BF16, tag="z2", name="z2")
                        nc.scalar.square(z2[:], zs[:, zbs])
                        nc.tensor.matmul(
                            ps_sq[:], ones16[:], z2[:],
                            start=(o == 0), stop=(o == 1), skip_group_check=True,
                        )
                    nc.scalar.copy(sc_sum[:, bs], ps_sum[:])
                    nc.scalar.copy(sc_sq[:, bs], ps_sq[:])
                nc.gpsimd.dma_start(st_loc[0:1, CW * c : CW * (c + 1)], sc_sum[:])
                nc.gpsimd.dma_start(st_loc[1:2, CW * c : CW * (c + 1)], sc_sq[:])

            def emit_pb_late(c):
                """stats finish + normalize for chunk c (needs the AllReduce)."""
                zs = zsb[c % 2]
                stt = tiny.tile([128, 2 * NPP], F32, tag="stt", name="stt")
                mu = tiny.tile([128, NPP], F32, tag="mu", name="mu")
                e2 = tiny.tile([128, NPP], F32, tag="e2", name="e2")
                m2 = tiny.tile([128, NPP], F32, tag="m2", name="m2")
                std = tiny.tile([128, NPP], F32, tag="std", name="std")
                inv = tiny.tile([128, NPP], BF16, tag="inv", name="inv")
                nms = tiny.tile([128, NPP], BF16, tag="nms", name="nms")
                ivr = tiny.tile([1, CW], BF16, tag="ivr", name="ivr")
                nmr = tiny.tile([1, CW], BF16, tag="nmr", name="nmr")
                nc.gpsimd.dma_start(stt[:, 0:NPP], st_glb[0:1, CW * c : CW * (c + 1)])
                nc.gpsimd.dma_start(stt[:, NPP : 2 * NPP],
                                    st_glb[1:2, CW * c : CW * (c + 1)])
                nc.vector.tensor_scalar_mul(mu[:], stt[:, 0:NPP], 1.0 / D)
                nc.vector.tensor_scalar_mul(e2[:], stt[:, NPP : 2 * NPP], 1.0 / D)
                nc.vector.tensor_mul(m2[:], mu[:], mu[:])
                nc.vector.tensor_sub(e2[:], e2[:], m2[:])
                nc.scalar.activation(std[:], e2[:], AF.Sqrt, bias=eps_t[:])
                with nc.allow_low_precision(reason="LN inv_std"):
                    nc.vector.reciprocal(inv[:], std[:])
                    nc.vector.scalar_tensor_tensor(nms[:], mu[:], -1.0, inv[:],
                                                   ALU.mult, ALU.mult)
                nc.gpsimd.dma_start(ivr[:], inv[:])
                nc.gpsimd.dma_start(nmr[:], nms[:])
                nc.gpsimd.partition_broadcast(invb[:], ivr[:])
                nc.gpsimd.partition_broadcast(nmsb[:], nmr[:])
                # out = (z*gamma)*inv + (nms*gamma) + beta
                for o in range(2):
                    for bj in range(NB):
                        bs = slice(1024 * bj, 1024 * (bj + 1))
                        zbs = slice(CW * o + 1024 * bj, CW * o + 1024 * (bj + 1))
                        t1 = scr.tile([128, 1024], BF16, tag="t1", name="t1")
                        t2 = scr.tile([128, 1024], BF16, tag="t2", name="t2")
                        ost = scr.tile([128, 1024], F32, tag="ost", name="ost")
                        nc.vector.scalar_tensor_tensor(
                            t1[:], zs[:, zbs], gsc[:, o : o + 1], invb[:, bs],
                            ALU.mult, ALU.mult)
                        nc.vector.scalar_tensor_tensor(
                            t2[:], nmsb[:, bs], gsc[:, o : o + 1], t1[:],
                            ALU.mult, ALU.add)
                        nc.scalar.activation(ost[:], t2[:], AF.Identity,
                                             bias=btc[:, o : o + 1])
                        nc.sync.dma_start(
                            og[128 * o : 128 * (o + 1),
                               CW * c + 1024 * bj : CW * c + 1024 * (bj + 1)],
                            ost[:])

            # ================= main loop =================
            for c in range(NCH):
                cbase = CW * c
                for lvl in range(DEPTH):
                    dil = 1 << lvl
                    for h in range(2):
                        # --- resolve input of this level ---
                        if lvl == 0:
                            src, tail, tw, src_full = None, None, 0, xf[h]
                        elif (lvl - 1) in FULL_A_LEVELS:
                            src, tail, tw, src_full = None, None, 0, af[(h, lvl - 1)]
                        else:
                            src = a_t[h][(lvl - 1) % 2]
                            tw = _tail_w(lvl - 1)
                            tail = ta[(h, lvl - 1)] if c > 0 else None
                            src_full = None
                        # --- output storage of this level's a-conv ---
                        if lvl in FULL_A_LEVELS:
                            a_out = af[(h, lvl)]
                            a_dst = a_out[:, cbase : cbase + CW]
                        else:
                            a_out = a_t[h][lvl % 2]
                            a_dst = a_out[:, 0:CW]

                        if lvl in FULL_A_LEVELS:
                            def evac_a(pp, cs, h=h, lvl=lvl, a_out=a_out,
                                       cbase=cbase):
                                nc.scalar.copy(
                                    a_out[:, cbase + cs.start : cbase + cs.stop],
                                    pp[:])
                                if lvl >= 1:
                                    nc.scalar.activation(sig[h][:, cs], pp[:],
                                                         AF.Sigmoid)
                        else:
                            def evac_a(pp, cs, h=h, lvl=lvl, a_out=a_out):
                                nc.scalar.copy(a_out[:, cs], pp[:])
                                if lvl >= 1:
                                    nc.scalar.activation(sig[h][:, cs], pp[:],
                                                         AF.Sigmoid)

                        _emit_conv_pe(nc, cps, evac_a, d0c[h], dil,
                                      src=src, tail=tail, tw=tw,
                                      src_full=src_full, cbase=cbase)
                        if (lvl < DEPTH - 1 and lvl not in FULL_A_LEVELS
                                and c < NCH - 1):
                            w = _tail_w(lvl)
                            nc.scalar.copy(ta[(h, lvl)][:], a_out[:, CW - w : CW])

                        if lvl >= 1:
                            b_prv = b_t[h][(lvl - 1) % 2]
                            nc.vector.tensor_mul(g_t[h][:], sig[h][:], b_prv[:])
                            if lvl == 1:
                                nc.vector.tensor_copy(y_t[h][:], g_t[h][:])
                            else:
                                nc.vector.tensor_add(y_t[h][:], y_t[h][:], g_t[h][:])

                        if lvl < DEPTH - 1:
                            b_cur = b_t[h][lvl % 2]
                            if lvl in DVE_B_LEVELS:
                                hh = h2c[h] if lvl == 0 else h1c[h]
                                _emit_conv_dve(nc, b_cur, hh, dil,
                                               src=src, tail=tail, tw=tw,
                                               src_full=src_full, cbase=cbase,
                                               tmp=tmp)
                            else:
                                def evac_b(pp, cs, b_cur=b_cur):
                                    nc.scalar.copy(b_cur[:, cs], pp[:])
                                _emit_conv_pe(nc, cps, evac_b, d1c[h], dil,
                                              src=src, tail=tail, tw=tw,
                                              src_full=src_full, cbase=cbase)

                for h in range(2):
                    nc.sync.dma_start(y_loc[c][128 * h : 128 * (h + 1), :],
                                      y_t[h][:])
                nc.gpsimd.collective_compute(
                    "AllGather", ALU.bypass, replica_groups=GROUPS,
                    ins=[y_loc[c].opt()], outs=[y_gat[c].opt()],
                )
            for c in range(NCH):
                emit_pb_early(c)
            nc.gpsimd.collective_compute(
                "AllReduce", ALU.add, replica_groups=GROUPS,
                ins=[st_loc.opt()], outs=[st_glb.opt()],
            )
            for c in range(NCH):
                emit_pb_late(c)

    nc.compile()
    return nc


def _get_program():
    if "nc" not in _CACHE:
        _CACHE["nc"] = _build_program()
    return _CACHE["nc"]


def _make_in_maps(inputs):
    x = np.ascontiguousarray(np.asarray(inputs["x"], dtype=np.float32))
    h0 = np.asarray(inputs["h0"], dtype=np.float32)[:, 0, :]  # [D, FS]
    h1 = np.asarray(inputs["h1"], dtype=np.float32)[:, 0, :]
    w = np.asarray(inputs["w_mix"], dtype=np.float32)
    bm = np.asarray(inputs["b_mix"], dtype=np.float32).reshape(D, 1)
    gm = np.asarray(inputs["ln_gamma"], dtype=np.float32).reshape(D, 1)
    bt = np.asarray(inputs["ln_beta"], dtype=np.float32).reshape(D, 1)

    wT16 = np.ascontiguousarray(w.T).astype(ml_dtypes.bfloat16)  # [c, o]

    in_maps = []
    for c in range(NC):
        beta, gamma = c // 4, c % 4
        cs = slice(CH * gamma, CH * (gamma + 1))
        h0c = h0[cs].astype(ml_dtypes.bfloat16)
        h1c = h1[cs].astype(ml_dtypes.bfloat16)
        d0m = np.zeros((2, FS, 128, 128), ml_dtypes.bfloat16)
        d1m = np.zeros((2, FS, 128, 128), ml_dtypes.bfloat16)
        for h in range(2):
            for k in range(FS):
                np.fill_diagonal(d0m[h, k], h0c[128 * h : 128 * (h + 1), k])
                np.fill_diagonal(d1m[h, k], h1c[128 * h : 128 * (h + 1), k])
        in_maps.append(
            {
                "xs16": np.ascontiguousarray(x[beta, cs, :]).astype(ml_dtypes.bfloat16),
                "d0": d0m,
                "d1": d1m,
                "h1x2": np.ascontiguousarray(2.0 * h1[cs]),
                "h1n": np.ascontiguousarray(h1[cs]),
                "wTs": np.ascontiguousarray(wT16[:, cs]),
                "bmixs": np.ascontiguousarray(bm[cs]),
                "gamc": np.ascontiguousarray(gm[cs]),
                "betc": np.ascontiguousarray(bt[cs]),
            }
        )
    return in_maps


def kernel(**inputs) -> np.ndarray:
    in_maps = _make_in_maps(inputs)
    nc = _get_program()
    res = run_bass_kernel_spmd(nc, in_maps, list(range(NC)))

    out = np.empty((B, D, L), dtype=np.float32)
    for c in range(NC):
        beta, gamma = c // 4, c % 4
        out[beta, CH * gamma : CH * (gamma + 1), :] = res.results[c]["og"]
    return out


# revision 10
# speedup vs baseline: 2.4032x; 1.2154x over previous
"""Trainium2 Bass kernel for nn_CustomMultiresLayer (B=2, D=1024, L=4096, FS=4).

Sharding (8 cores): core c -> batch beta=c//4, channel shard gamma=c%4
(256 channels). The multires tree + gated combination run position-chunked
(NCH=2 chunks of CW columns): the first chunk's AllGather of the gated
tensor y overlaps the second chunk's tree compute, and the channel-mix /
LayerNorm stats of chunk 0 overlap chunk 1's AllGather. A single bf16
AllReduce carries both chunks' LayerNorm partial sums (collective count
kept at 3 -- more collectives inflate the NEFF's startup barrier).

Engine plan (bf16 tree): conv taps are diagonal-weight matmuls on PE,
ACT evacuates the a-chain + sigmoids + PE-b-convs, DVE computes the
gating and roughly half the b-convs directly (tensor_scalar +
shifted-add pairs). Causality makes chunking exact: each level keeps a
tail of its output (full-width buffer for the deepest level) for the
next chunk's dilated convs. LayerNorm stats finish in a [128,CW/128]
layout (128-lane reciprocal) and are partition-broadcast for a
PSUM-free DVE normalize. Emission is split early/late so no engine
queue blocks tree work behind collective-dependent ops.
"""

import numpy as np
import ml_dtypes

import concourse.bacc as bacc
import concourse.mybir as mybir
import concourse.tile as tile
from concourse.bass_utils import run_bass_kernel_spmd

F32 = mybir.dt.float32
BF16 = mybir.dt.bfloat16
AF = mybir.ActivationFunctionType
ALU = mybir.AluOpType

B, D, L = 2, 1024, 4096
FS, DEPTH = 4, 11
LN_EPS = 1e-5
NC = 8
CH = 256          # channels per core (2 half-tiles of 128)
NMM = 512         # PE moving-dim limit
NCH = 2           # position chunks
CW = L // NCH     # chunk width
NPP = CW // 128   # stats cols per partition
NB = CW // 1024   # 1024-col blocks per chunk (norm granularity)
GROUPS = [[0, 1, 2, 3], [4, 5, 6, 7]]

# levels whose a-output must stay full-width (next level's shift exceeds CW)
FULL_A_LEVELS = frozenset(l for l in range(DEPTH - 1) if 3 * (1 << (l + 1)) > CW)
# b-conv levels computed on DVE instead of PE (level 0 always: doubled h1)
DVE_B_LEVELS = frozenset({0, 4, 6})

_CACHE = {}


def _tail_w(lvl):
    """Tail width kept from level `lvl`'s a-output for the next chunk."""
    return min(3 * (1 << (lvl + 1)), CW)


def _emit_conv_pe(nc, cps, evac, diag, dil, src=None, tail=None, tw=0,
                  src_full=None, cbase=0):
    """4-tap dilated causal depthwise conv for one chunk, tap-outer per
    1024-col PSUM tile. Either src ([128,CW] chunk tile, + optional tail
    [128,tw]) or src_full ([128,L] tile read at global offset cbase).
    evac(pp, cs) evacuates one PSUM tile covering chunk-local slice cs."""
    PW = 1024
    nsub = PW // NMM
    for p0 in range(0, CW, PW):
        pp = cps.tile([128, PW], F32, tag="cps", name="cps")
        pieces = [[] for _ in range(nsub)]
        for k in (3, 2, 1, 0):
            s = (3 - k) * dil
            for bi in range(nsub):
                m0 = p0 + NMM * bi          # chunk-local block offset
                if src_full is not None:
                    base = cbase + m0
                    lo = max(0, s - base)
                    if lo < NMM:
                        pieces[bi].append((k, s, "full", lo, NMM))
                else:
                    if m0 < s and tail is not None:
                        hi = min(NMM, s - m0)
                        pieces[bi].append((k, s, "tail", 0, hi))
                    lo = max(0, s - m0)
                    if lo < NMM:
                        pieces[bi].append((k, s, "main", lo, NMM))
        for k in (3, 2, 1, 0):
            for bi in range(nsub):
                m0 = p0 + NMM * bi
                po = NMM * bi               # offset within pp
                plist = pieces[bi]
                for pi, (pk, s, kind, lo, hi) in enumerate(plist):
                    if pk != k:
                        continue
                    if kind == "full":
                        rhs = src_full[:, cbase + m0 + lo - s : cbase + m0 + NMM - s]
                    elif kind == "main":
                        rhs = src[:, m0 + lo - s : m0 + NMM - s]
                    else:
                        rhs = tail[:, tw - s + m0 : tw - s + m0 + hi]
                    nc.tensor.matmul(
                        pp[:, po + lo : po + hi],
                        diag[:, 128 * k : 128 * (k + 1)],
                        rhs,
                        start=(pi == 0),
                        stop=(pi == len(plist) - 1),
                    )
        evac(pp, slice(p0, p0 + PW))


def _emit_conv_dve(nc, dst, h, dil, src=None, tail=None, tw=0,
                   src_full=None, cbase=0, tmp=None):
    """4-tap conv on DVE for one chunk: tensor_scalar tap-3, then either
    tensor_scalar+shifted-add pairs (even s, 2x mode) or STT."""
    if src_full is not None:
        nc.vector.tensor_scalar_mul(dst[:], src_full[:, cbase : cbase + CW], h[:, 3:4])
    else:
        nc.vector.tensor_scalar_mul(dst[:], src[:], h[:, 3:4])
    for k in (2, 1, 0):
        s = (3 - k) * dil
        if src_full is not None:
            lo = max(0, s - cbase)
            if lo < CW:
                nc.vector.scalar_tensor_tensor(
                    dst[:, lo:CW],
                    src_full[:, cbase + lo - s : cbase + CW - s],
                    h[:, k : k + 1], dst[:, lo:CW], ALU.mult, ALU.add,
                )
            continue
        if s < CW and tmp is not None and s % 2 == 0:
            nc.vector.tensor_scalar_mul(tmp[:, 0 : CW - s], src[:, 0 : CW - s],
                                        h[:, k : k + 1])
            nc.vector.tensor_add(dst[:, s:CW], dst[:, s:CW], tmp[:, 0 : CW - s])
        elif s < CW:
            nc.vector.scalar_tensor_tensor(
                dst[:, s:CW], src[:, 0 : CW - s], h[:, k : k + 1],
                dst[:, s:CW], ALU.mult, ALU.add,
            )
        if tail is not None and s > 0:
            nc.vector.scalar_tensor_tensor(
                dst[:, 0 : min(s, CW)], tail[:, tw - s : tw - s + min(s, CW)],
                h[:, k : k + 1], dst[:, 0 : min(s, CW)], ALU.mult, ALU.add,
            )


def _build_program():
    nc = bacc.Bacc("TRN2", target_bir_lowering=False, debug=False, num_devices=NC)

    xs16 = nc.dram_tensor("xs16", [CH, L], BF16, kind="ExternalInput").ap()
    d0 = nc.dram_tensor("d0", [2, FS, 128, 128], BF16, kind="ExternalInput").ap()
    d1 = nc.dram_tensor("d1", [2, FS, 128, 128], BF16, kind="ExternalInput").ap()
    h1x2 = nc.dram_tensor("h1x2", [CH, FS], F32, kind="ExternalInput").ap()
    h1n = nc.dram_tensor("h1n", [CH, FS], F32, kind="ExternalInput").ap()
    wTs = nc.dram_tensor("wTs", [D, CH], BF16, kind="ExternalInput").ap()
    bmixs = nc.dram_tensor("bmixs", [CH, 1], F32, kind="ExternalInput").ap()
    gamc = nc.dram_tensor("gamc", [CH, 1], F32, kind="ExternalInput").ap()
    betc = nc.dram_tensor("betc", [CH, 1], F32, kind="ExternalInput").ap()
    og = nc.dram_tensor("og", [CH, L], F32, kind="ExternalOutput").ap()

    with tile.TileContext(nc) as tc:
        with (
            tc.tile_pool(name="dram", bufs=1, space="DRAM") as dram,
            tc.tile_pool(name="smalls", bufs=1) as smalls,
            tc.tile_pool(name="tree", bufs=1) as tp,
            tc.tile_pool(name="cpsum", bufs=2, space="PSUM") as cps,
            tc.tile_pool(name="mix", bufs=1) as mx,
            tc.tile_pool(name="scr", bufs=2) as scr,
            tc.tile_pool(name="tiny", bufs=1) as tiny,
            tc.tile_pool(name="mmps", bufs=2, space="PSUM") as psmm,
            tc.tile_pool(name="stps", bufs=1, space="PSUM") as psst,
        ):
            y_loc = [dram.tile([CH, CW], BF16, name=f"yloc{c}") for c in range(NCH)]
            y_gat = [dram.tile([D, CW], BF16, name=f"ygat{c}") for c in range(NCH)]
            st_loc = dram.tile([2, L], BF16, name="stloc")
            st_glb = dram.tile([2, L], BF16, name="stglb")

            # ---- persistent loads ----
            wsb = smalls.tile([128, 8 * CH], BF16, tag="wsb", name="wsb")
            xf = [smalls.tile([128, L], BF16, tag=f"xf{h}", name=f"xf{h}") for h in range(2)]
            h2c = [smalls.tile([128, FS], F32, tag=f"h2c{h}", name=f"h2c{h}") for h in range(2)]
            h1c = [smalls.tile([128, FS], F32, tag=f"h1c{h}", name=f"h1c{h}") for h in range(2)]
            d0c = [smalls.tile([128, FS * 128], BF16, tag=f"d0c{h}", name=f"d0c{h}") for h in range(2)]
            d1c = [smalls.tile([128, FS * 128], BF16, tag=f"d1c{h}", name=f"d1c{h}") for h in range(2)]
            bsc = smalls.tile([128, 2], F32, tag="bsc", name="bsc")
            gsc = smalls.tile([128, 2], F32, tag="gsc", name="gsc")
            btc = smalls.tile([128, 2], F32, tag="btc", name="btc")
            ones16 = smalls.tile([128, 1], BF16, tag="ones16", name="ones16")
            eps_t = smalls.tile([128, 1], F32, tag="eps", name="eps")

            for k in range(8):
                nc.sync.dma_start(wsb[:, CH * k : CH * (k + 1)],
                                  wTs[128 * k : 128 * (k + 1), :])
            for h in range(2):
                rs = slice(128 * h, 128 * (h + 1))
                nc.sync.dma_start(xf[h][:], xs16[rs, :])
                nc.sync.dma_start(h2c[h][:], h1x2[rs, :])
                nc.sync.dma_start(h1c[h][:], h1n[rs, :])
                for k in range(FS):
                    ks = slice(128 * k, 128 * (k + 1))
                    nc.sync.dma_start(d0c[h][:, ks], d0[h, k])
                    nc.sync.dma_start(d1c[h][:, ks], d1[h, k])
                nc.sync.dma_start(bsc[:, h : h + 1], bmixs[rs, :])
                nc.sync.dma_start(gsc[:, h : h + 1], gamc[rs, :])
                nc.sync.dma_start(btc[:, h : h + 1], betc[rs, :])
            with tc.tile_pool(name="stage0", bufs=1) as st0:
                o32 = st0.tile([128, 1], F32, tag="o32", name="o32")
                nc.vector.memset(o32[:], 1.0)
                nc.vector.tensor_copy(ones16[:], o32[:])
                nc.vector.memset(eps_t[:], LN_EPS)

            # ---- tree tiles ----
            a_t = [[tp.tile([128, CW], BF16, tag=f"a{h}{i}", name=f"a{h}{i}")
                    for i in range(2)] for h in range(2)]
            af = {(h, l): tp.tile([128, L], BF16, tag=f"af{h}{l}", name=f"af{h}{l}")
                  for h in range(2) for l in FULL_A_LEVELS}
            ta = {(h, l): tp.tile([128, _tail_w(l)], BF16, tag=f"ta{h}{l}",
                                  name=f"ta{h}{l}")
                  for h in range(2) for l in range(DEPTH - 1) if l not in FULL_A_LEVELS}
            b_t = [[tp.tile([128, CW], BF16, tag=f"b{h}{i}", name=f"b{h}{i}")
                    for i in range(2)] for h in range(2)]
            sig = [tp.tile([128, CW], BF16, tag=f"s{h}", name=f"s{h}") for h in range(2)]
            g_t = [tp.tile([128, CW], BF16, tag=f"g{h}", name=f"g{h}") for h in range(2)]
            y_t = [tp.tile([128, CW], BF16, tag=f"y{h}", name=f"y{h}") for h in range(2)]
            tmp = tp.tile([128, CW], BF16, tag="tmp", name="tmp")

            # ---- phase-B tiles ----
            yhs = [mx.tile([128, CW], BF16, tag=f"yh{k}", name=f"yh{k}") for k in range(8)]
            zsb = [mx.tile([128, 2 * CW], BF16, tag=f"zsb{p}", name=f"zsb{p}")
                   for p in range(2)]
            invb = mx.tile([128, CW], BF16, tag="invb", name="invb")
            nmsb = mx.tile([128, CW], BF16, tag="nmsb", name="nmsb")

            def emit_yh_loads(c):
                for k in range(8):
                    nc.sync.dma_start(yhs[k][:], y_gat[c][128 * k : 128 * (k + 1), :])

            def emit_pb_early(c):
                """mix + z + LN partial sums for chunk c (needs AG_c only)."""
                zs = zsb[c % 2]
                sc_sum = tiny.tile([1, CW], BF16, tag="scs", name="scs")
                sc_sq = tiny.tile([1, CW], BF16, tag="scq", name="scq")
                for o in range(2):
                    for bi in range(CW // NMM):
                        bs = slice(NMM * bi, NMM * (bi + 1))
                        zbs = slice(CW * o + NMM * bi, CW * o + NMM * (bi + 1))
                        pm = psmm.tile([128, NMM], F32, tag="pm", name="pm")
                        for k in range(8):
                            nc.tensor.matmul(
                                pm[:],
                                wsb[:, CH * k + 128 * o : CH * k + 128 * (o + 1)],
                                yhs[k][:, bs],
                                start=(k == 0), stop=(k == 7),
                            )
                        nc.vector.scalar_tensor_tensor(
                            zs[:, zbs], pm[:], bsc[:, o : o + 1],
                            xf[o][:, CW * c + NMM * bi : CW * c + NMM * (bi + 1)],
                            ALU.add, ALU.add,
                        )
                for bi in range(CW // NMM):
                    bs = slice(NMM * bi, NMM * (bi + 1))
                    ps_sum = psst.tile([1, NMM], F32, tag="sts", name="sts")
                    ps_sq = psst.tile([1, NMM], F32, tag="stq", name="stq")
                    for o in range(2):
                        zbs = slice(CW * o + NMM * bi, CW * o + NMM * (bi + 1))
                        nc.tensor.matmul(
                            ps_sum[:], ones16[:], zs[:, zbs],
                            start=(o == 0), stop=(o == 1), skip_group_check=True,
                        )
                        z2 = scr.tile([128, NMM], BF16, tag="z2", name="z2")
                        nc.scalar.square(z2[:], zs[:, zbs])
                        nc.tensor.matmul(
                            ps_sq[:], ones16[:], z2[:],
                            start=(o == 0), stop=(o == 1), skip_group_check=True,
                        )
                    nc.scalar.copy(sc_sum[:, bs], ps_sum[:])
                    nc.scalar.copy(sc_sq[:, bs], ps_sq[:])
                nc.gpsimd.dma_start(st_loc[0:1, CW * c : CW * (c + 1)], sc_sum[:])
                nc.gpsimd.dma_start(st_loc[1:2, CW * c : CW * (c + 1)], sc_sq[:])

            def emit_pb_late(c):
                """stats finish + normalize for chunk c (needs the AllReduce)."""
                zs = zsb[c % 2]
                stt = tiny.tile([128, 2 * NPP], F32, tag="stt", name="stt")
                mu = tiny.tile([128, NPP], F32, tag="mu", name="mu")
                e2 = tiny.tile([128, NPP], F32, tag="e2", name="e2")
                m2 = tiny.tile([128, NPP], F32, tag="m2", name="m2")
                std = tiny.tile([128, NPP], F32, tag="std", name="std")
                inv = tiny.tile([128, NPP], BF16, tag="inv", name="inv")
                nms = tiny.tile([128, NPP], BF16, tag="nms", name="nms")
                ivr = tiny.tile([1, CW], BF16, tag="ivr", name="ivr")
                nmr = tiny.tile([1, CW], BF16, tag="nmr", name="nmr")
                nc.gpsimd.dma_start(stt[:, 0:NPP], st_glb[0:1, CW * c : CW * (c + 1)])
                nc.gpsimd.dma_start(stt[:, NPP : 2 * NPP],
                                    st_glb[1:2, CW * c : CW * (c + 1)])
                nc.vector.tensor_scalar_mul(mu[:], stt[:, 0:NPP], 1.0 / D)
                nc.vector.tensor_scalar_mul(e2[:], stt[:, NPP : 2 * NPP], 1.0 / D)
                nc.vector.tensor_mul(m2[:], mu[:], mu[:])
                nc.vector.tensor_sub(e2[:], e2[:], m2[:])
                nc.scalar.activation(std[:], e2[:], AF.Sqrt, bias=eps_t[:])
                with nc.allow_low_precision(reason="LN inv_std"):
                    nc.vector.reciprocal(inv[:], std[:])
                    nc.vector.scalar_tensor_tensor(nms[:], mu[:], -1.0, inv[:],
                                                   ALU.mult, ALU.mult)
                nc.gpsimd.dma_start(ivr[:], inv[:])
                nc.gpsimd.dma_start(nmr[:], nms[:])
                nc.gpsimd.partition_broadcast(invb[:], ivr[:])
                nc.gpsimd.partition_broadcast(nmsb[:], nmr[:])
                # out = (z*gamma)*inv + (nms*gamma) + beta
                for o in range(2):
                    for bj in range(NB):
                        bs = slice(1024 * bj, 1024 * (bj + 1))
                        zbs = slice(CW * o + 1024 * bj, CW * o + 1024 * (bj + 1))
                        t1 = scr.tile([128, 1024], BF16, tag="t1", name="t1")
                        t2 = scr.tile([128, 1024], BF16, tag="t2", name="t2")
                        ost = scr.tile([128, 1024], F32, tag="ost", name="ost")
                        nc.vector.scalar_tensor_tensor(
                            t1[:], zs[:, zbs], gsc[:, o : o + 1], invb[:, bs],
                            ALU.mult, ALU.mult)
                        nc.vector.scalar_tensor_tensor(
                            t2[:], nmsb[:, bs], gsc[:, o : o + 1], t1[:],
                            ALU.mult, ALU.add)
                        nc.scalar.activation(ost[:], t2[:], AF.Identity,
                                             bias=btc[:, o : o + 1])
                        nc.sync.dma_start(
                            og[128 * o : 128 * (o + 1),
                               CW * c + 1024 * bj : CW * c + 1024 * (bj + 1)],
                            ost[:])

            # ================= main loop =================
            for c in range(NCH):
                cbase = CW * c
                for lvl in range(DEPTH):
                    dil = 1 << lvl
                    for h in range(2):
                        # --- resolve input of this level ---
                        if lvl == 0:
                            src, tail, tw, src_full = None, None, 0, xf[h]
                        elif (lvl - 1) in FULL_A_LEVELS:
                            src, tail, tw, src_full = None, None, 0, af[(h, lvl - 1)]
                        else:
                            src = a_t[h][(lvl - 1) % 2]
                            tw = _tail_w(lvl - 1)
                            tail = ta[(h, lvl - 1)] if c > 0 else None
                            src_full = None
                        # --- output storage of this level's a-conv ---
                        if lvl in FULL_A_LEVELS:
                            a_out = af[(h, lvl)]
                            a_dst = a_out[:, cbase : cbase + CW]
                        else:
                            a_out = a_t[h][lvl % 2]
                            a_dst = a_out[:, 0:CW]

                        if lvl in FULL_A_LEVELS:
                            def evac_a(pp, cs, h=h, lvl=lvl, a_out=a_out,
                                       cbase=cbase):
                                nc.scalar.copy(
                                    a_out[:, cbase + cs.start : cbase + cs.stop],
                                    pp[:])
                                if lvl >= 1:
                                    nc.scalar.activation(sig[h][:, cs], pp[:],
                                                         AF.Sigmoid)
                        else:
                            def evac_a(pp, cs, h=h, lvl=lvl, a_out=a_out):
                                nc.scalar.copy(a_out[:, cs], pp[:])
                                if lvl >= 1:
                                    nc.scalar.activation(sig[h][:, cs], pp[:],
                                                         AF.Sigmoid)

                        _emit_conv_pe(nc, cps, evac_a, d0c[h], dil,
                                      src=src, tail=tail, tw=tw,
                                      src_full=src_full, cbase=cbase)
                        if (lvl < DEPTH - 1 and lvl not in FULL_A_LEVELS
                                and c < NCH - 1):
                            w = _tail_w(lvl)
                            nc.scalar.copy(ta[(h, lvl)][:], a_out[:, CW - w : CW])

                        if lvl >= 1:
                            b_prv = b_t[h][(lvl - 1) % 2]
                            nc.vector.tensor_mul(g_t[h][:], sig[h][:], b_prv[:])
                            if lvl == 1:
                                nc.vector.tensor_copy(y_t[h][:], g_t[h][:])
                            else:
                                nc.vector.tensor_add(y_t[h][:], y_t[h][:], g_t[h][:])

                        if lvl < DEPTH - 1:
                            b_cur = b_t[h][lvl % 2]
                            if lvl in DVE_B_LEVELS:
                                hh = h2c[h] if lvl == 0 else h1c[h]
                                _emit_conv_dve(nc, b_cur, hh, dil,
                                               src=src, tail=tail, tw=tw,
                                               src_full=src_full, cbase=cbase,
                                               tmp=tmp)
                            else:
                                def evac_b(pp, cs, b_cur=b_cur):
                                    nc.scalar.copy(b_cur[:, cs], pp[:])
                                _emit_conv_pe(nc, cps, evac_b, d1c[h], dil,
                                              src=src, tail=tail, tw=tw,
                                              src_full=src_full, cbase=cbase)

                for h in range(2):
                    nc.sync.dma_start(y_loc[c][128 * h : 128 * (h + 1), :],
                                      y_t[h][:])
                nc.gpsimd.collective_compute(
                    "AllGather", ALU.bypass, replica_groups=GROUPS,
                    ins=[y_loc[c].opt()], outs=[y_gat[c].opt()],
                )
            emit_yh_loads(0)
            tc.no_sync_barrier()
            emit_pb_early(0)
            tc.no_sync_barrier()
            emit_yh_loads(1)
            emit_pb_early(1)
            nc.gpsimd.collective_compute(
                "AllReduce", ALU.add, replica_groups=GROUPS,
                ins=[st_loc.opt()], outs=[st_glb.opt()],
            )
            tc.no_sync_barrier()
            for c in range(NCH):
                emit_pb_late(c)

    nc.compile()
    return nc


def _get_program():
    if "nc" not in _CACHE:
        _CACHE["nc"] = _build_program()
    return _CACHE["nc"]


def _make_in_maps(inputs):
    x = np.ascontiguousarray(np.asarray(inputs["x"], dtype=np.float32))
    h0 = np.asarray(inputs["h0"], dtype=np.float32)[:, 0, :]  # [D, FS]
    h1 = np.asarray(inputs["h1"], dtype=np.float32)[:, 0, :]
    w = np.asarray(inputs["w_mix"], dtype=np.float32)
    bm = np.asarray(inputs["b_mix"], dtype=np.float32).reshape(D, 1)
    gm = np.asarray(inputs["ln_gamma"], dtype=np.float32).reshape(D, 1)
    bt = np.asarray(inputs["ln_beta"], dtype=np.float32).reshape(D, 1)

    wT16 = np.ascontiguousarray(w.T).astype(ml_dtypes.bfloat16)  # [c, o]

    in_maps = []
    for c in range(NC):
        beta, gamma = c // 4, c % 4
        cs = slice(CH * gamma, CH * (gamma + 1))
        h0c = h0[cs].astype(ml_dtypes.bfloat16)
        h1c = h1[cs].astype(ml_dtypes.bfloat16)
        d0m = np.zeros((2, FS, 128, 128), ml_dtypes.bfloat16)
        d1m = np.zeros((2, FS, 128, 128), ml_dtypes.bfloat16)
        for h in range(2):
            for k in range(FS):
                np.fill_diagonal(d0m[h, k], h0c[128 * h : 128 * (h + 1), k])
                np.fill_diagonal(d1m[h, k], h1c[128 * h : 128 * (h + 1), k])
        in_maps.append(
            {
                "xs16": np.ascontiguousarray(x[beta, cs, :]).astype(ml_dtypes.bfloat16),
                "d0": d0m,
                "d1": d1m,
                "h1x2": np.ascontiguousarray(2.0 * h1[cs]),
                "h1n": np.ascontiguousarray(h1[cs]),
                "wTs": np.ascontiguousarray(wT16[:, cs]),
                "bmixs": np.ascontiguousarray(bm[cs]),
                "gamc": np.ascontiguousarray(gm[cs]),
                "betc": np.ascontiguousarray(bt[cs]),
            }
        )
    return in_maps


def kernel(**inputs) -> np.ndarray:
    in_maps = _make_in_maps(inputs)
    nc = _get_program()
    res = run_bass_kernel_spmd(nc, in_maps, list(range(NC)))

    out = np.empty((B, D, L), dtype=np.float32)
    for c in range(NC):
        beta, gamma = c // 4, c % 4
        out[beta, CH * gamma : CH * (gamma + 1), :] = res.results[c]["og"]
    return out
